# revision 20
# baseline (speedup 1.0000x reference)
"""Trainium2 Bass kernel for nn_AblationGCN (2-layer OGB-style GCN).

Strategy (v4): destination-node sharding with on-chip one-hot scatter
matrix generation.

  norm_e = dis[src]*dis[dst] factorizes: the gather table holds
  xs = dis*relu(h) (dis[src] rides along with the gathered row). The
  dis[dst] factor is constant per output row, so the scatter matrix S is
  PURE 0/1 one-hot: it is generated on-chip, one DVE tensor_scalar
  (iota == dst_pos) per 128-edge chunk, from a tiny [P, NCH] dst-pos
  table (~0.2 MB) instead of streaming 26 MB of fp16 S per layer.
  Messages aggregate as
      psum[slot, f] += S_chunk^T @ xs_chunk
  The missing dis[dst] row scale commutes with LayerNorm up to the
  epsilon: LN(dis*x) == (x-mu(x))*rsqrt(var(x) + eps*deg), so the Sqrt
  bias uses a per-slot eps*deg column and the result is exact. The root
  term (pre-divided by dis[dst] so it lives in the same scaled basis) is
  added into PSUM via an identity matmul, LayerNorm fused on
  Scalar/Vector.

  Nodes are packed into 8 cores x 98 blocks of 128 slots. A source's window
  (= its core pair, 25088 contiguous table rows) fits int16 gather indices.
  Each block has 4 window cells, capacity-balanced by the packer (256 edges
  for 92 "small" blocks, 384 for 6 "big" blocks) so padding is ~3%.
  Gathers run as 1024-idx GPSIMD SWDGE calls, 4 queues round-robin.

  h1 stays in SBUF; the layer-1 root terms are precomputed during the
  xs1 AllGather window. All per-block DRAM IO is batched per group.
"""
import os
import numpy as np

import concourse.bass as bass
import concourse.bacc as bacc
import concourse.mybir as mybir
import concourse.tile as tile
from concourse.bass_utils import run_bass_kernel_spmd

P = 128
D = 128
NCORES = 8
LN_EPS = 1e-5
B = 98                   # blocks per core
SH = B * P               # 12544 slots per core
NBIG = 6                 # big blocks (cells of 384) per core
NSMALL = B - NBIG        # small blocks (cells of 256)
CBIG = 3                 # chunks per big cell
CSMALL = 2               # chunks per small cell
NWIN = 4                 # windows (= 4 cores x shard half)
GBIG = 2                 # big blocks per gather group (2*384 = 768 idx)
GSMALL = 4               # small blocks per group (4*256 = 1024 idx)
dt = mybir.dt

SLOTS = NCORES * SH      # 100352
WROWS = SLOTS // NWIN    # 25088 rows per window
HBLK = 49                # blocks per shard half
HROWS = HBLK * P         # 6272 rows per half-shard

# groups (block partition for gather calls); halves are group-aligned:
# half0 = blocks 0..48 (6 big + 43 small), half1 = blocks 49..97 (49 small)
GROUPS = ([(GBIG, CBIG)] * (NBIG // GBIG)      # 3 groups: blocks 0-5
          + [(GSMALL, CSMALL)] * 10            # blocks 6-45
          + [(3, CSMALL)]                      # blocks 46-48
          + [(GSMALL, CSMALL)] * 12            # blocks 49-96
          + [(1, CSMALL)])                     # block 97
NGRP_H0 = 14             # groups in half 0
NCH = NBIG * NWIN * CBIG + NSMALL * NWIN * CSMALL          # S chunks/core
TOTIDX = NCH * P                                            # padded edges

_CACHE = {}


# --------------------------------------------------------------------------
# Host-side planning
# --------------------------------------------------------------------------

def _assign_cores(deg_in, n):
    """Assign nodes to cores, balancing in-edge load, <=SH nodes per core."""
    import heapq
    order = np.argsort(-deg_in, kind="stable")
    core_of = np.full(n, -1, np.int32)
    cnt = np.zeros(NCORES, np.int64)
    heap = [(0, c) for c in range(NCORES)]
    heapq.heapify(heap)
    for v in order:
        stash = []
        while True:
            load, c = heapq.heappop(heap)
            if cnt[c] < SH:
                core_of[v] = c
                cnt[c] += 1
                heapq.heappush(heap, (load + int(deg_in[v]), c))
                break
            stash.append((load, c))
        for s in stash:
            heapq.heappush(heap, s)
    return core_of


def _split_half(nodes, deg_in):
    """Split a core's nodes into half0 (<= HBLK*P, higher capacity: big
    bins) and half1 (<= HBLK*P), balancing in-edge load against capacity."""
    cap0 = NBIG * CBIG * P + (HBLK - NBIG) * CSMALL * P   # per window
    cap1 = HBLK * CSMALL * P
    order = nodes[np.argsort(-deg_in[nodes], kind="stable")]
    h = np.zeros(len(order), np.int8)
    l0 = l1 = 0
    c0 = c1 = 0
    for i, v in enumerate(order):
        d = int(deg_in[v])
        pick0 = (l0 + d) * cap1 <= (l1 + d) * cap0
        if c0 >= HBLK * P:
            pick0 = False
        if c1 >= HBLK * P:
            pick0 = True
        if pick0:
            h[i] = 0
            l0 += d
            c0 += 1
        else:
            h[i] = 1
            l1 += d
            c1 += 1
    half_of = {}
    for i, v in enumerate(order):
        half_of[v] = int(h[i])
    return half_of


def _pack_half(nodes, dmat, nbig, nbins):
    """Pack nodes (per-window in-degree rows dmat[v]) into nbins bins:
    first nbig bins with per-cell cap CBIG*P, rest CSMALL*P, <=128 each."""
    nb = nbins
    cap = np.empty((nb, NWIN), np.int64)
    cap[:nbig] = CBIG * P
    cap[nbig:] = CSMALL * P
    load = np.zeros((nb, NWIN), np.int64)
    cnt = np.zeros(nb, np.int32)
    tot = dmat[nodes].sum(axis=1)
    order = nodes[np.argsort(-tot, kind="stable")]
    bin_of = {}
    spare = []
    for v in order:
        d = dmat[v]
        fit = ((load + d) <= cap).all(axis=1) & (cnt < P)
        idx = np.nonzero(fit)[0]
        if idx.size == 0:
            spare.append(v)
            continue
        rem = (cap[idx] - load[idx] - d).min(axis=1)
        b = idx[np.argmax(rem)]
        bin_of[v] = b
        load[b] += d
        cnt[b] += 1
    for v in spare:
        d = dmat[v]
        placed = False
        for b in np.argsort((cap - load).min(axis=1))[::-1]:
            if cnt[b] < P and ((load[b] + d) <= cap[b]).all():
                bin_of[v] = b
                load[b] += d
                cnt[b] += 1
                placed = True
                break
        if placed:
            continue
        for b in range(nb):
            members = [u for u, bb in bin_of.items() if bb == b]
            done = False
            for u in members:
                du = dmat[u]
                if not ((load[b] - du + d) <= cap[b]).all():
                    continue
                for b2 in range(nb):
                    if b2 == b or cnt[b2] >= P:
                        continue
                    if ((load[b2] + du) <= cap[b2]).all():
                        bin_of[u] = b2
                        load[b2] += du
                        cnt[b2] += 1
                        load[b] -= du
                        cnt[b] -= 1
                        bin_of[v] = b
                        load[b] += d
                        cnt[b] += 1
                        done = True
                        break
                if done:
                    break
            if done:
                placed = True
                break
        if not placed:
            return None, False
    return bin_of, True


def _plan(edge_index, n, e):
    row = np.asarray(edge_index[0], dtype=np.int64)
    col = np.asarray(edge_index[1], dtype=np.int64)

    deg = np.bincount(row, minlength=n).astype(np.float64) + 1.0
    dis = deg ** -0.5
    deginv = 1.0 / deg
    deg_in = np.bincount(col, minlength=n)

    core_of = _assign_cores(deg_in, n)

    # shard-half assignment (window = 2*half + core//4)
    half_of = np.zeros(n, np.int8)
    for c in range(NCORES):
        nodes = np.nonzero(core_of == c)[0]
        hm = _split_half(nodes, deg_in)
        for v, h in hm.items():
            half_of[v] = h
    win_of = (2 * half_of.astype(np.int32) + core_of // 4)

    dmat = np.zeros((n, NWIN), np.int64)
    np.add.at(dmat, (col, win_of[row]), 1)

    block_of = np.full(n, -1, np.int32)
    pos_of = np.full(n, -1, np.int32)
    for c in range(NCORES):
        for h in (0, 1):
            nodes = np.nonzero((core_of == c) & (half_of == h))[0]
            nbig = NBIG if h == 0 else 0
            bin_of, ok = _pack_half(nodes, dmat, nbig, HBLK)
            assert ok, f"packing failed for core {c} half {h}"
            byb = [[] for _ in range(HBLK)]
            for v, b in bin_of.items():
                byb[b].append(v)
            for b in range(HBLK):
                for i, v in enumerate(byb[b]):
                    block_of[v] = h * HBLK + b
                    pos_of[v] = i

    slot = (core_of.astype(np.int64) * SH + block_of.astype(np.int64) * P
            + pos_of.astype(np.int64))
    # gather-table row: [c0h0|..|c7h0|c0h1|..|c7h1], halves of HROWS rows
    lslot = block_of.astype(np.int64) * P + pos_of.astype(np.int64)
    trow = (half_of.astype(np.int64) * (NCORES * HROWS)
            + core_of.astype(np.int64) * HROWS
            + (lslot - half_of.astype(np.int64) * HROWS))

    src_slot = trow[row]
    w_e = (src_slot // WROWS).astype(np.int64)
    dst_core = core_of[col]
    dst_block = block_of[col]
    dst_pos = pos_of[col]

    ekey = np.lexsort((src_slot, w_e, dst_block, dst_core))
    rs, ws = src_slot[ekey], w_e[ekey]
    dcs, dbs, dps = dst_core[ekey], dst_block[ekey], dst_pos[ekey]

    cellcap = np.where(np.arange(B) < NBIG, CBIG * P, CSMALL * P)

    gsizes = []
    b0 = 0
    for (G, C) in GROUPS:
        gsizes.append((G, C, list(range(b0, b0 + G))))
        b0 += G
    assert b0 == B

    # chunk linear order: g-major, then w, j, block-in-group
    chunk_idx = {}
    k = 0
    for gi, (G, C, blocks) in enumerate(gsizes):
        for w in range(NWIN):
            for j in range(C):
                for bi in range(G):
                    chunk_idx[(blocks[bi], w, j)] = k
                    k += 1
    assert k == NCH

    cellcnt = np.zeros((NCORES, B, NWIN), np.int64)
    np.add.at(cellcnt, (dcs, dbs, ws), 1)
    assert (cellcnt <= cellcap[None, :, None]).all(), "cell overflow"

    cell_key = (dcs * B + dbs) * NWIN + ws
    first = np.zeros(NCORES * B * NWIN + 1, np.int64)
    np.add.at(first, cell_key + 1, 1)
    first = np.cumsum(first)
    rank = np.arange(e) - first[cell_key]

    j_e = rank // P
    p_e = rank % P
    ctab = np.full((B, NWIN, CBIG), -1, np.int64)
    for (b, w, j), kk in chunk_idx.items():
        ctab[b, w, j] = kk
    ck = ctab[dbs, ws, j_e]
    assert (ck >= 0).all()

    # dst-pos table: lane p of chunk k scatters to column dpos[p, k].
    # 200 (outside 0..P-1) marks unused lanes -> all-zero one-hot column.
    dpos = np.full((NCORES, P, NCH), 200.0, np.float32)
    dpos[dcs, p_e, ck] = dps

    # gather idx per call (cells contiguous per call), wrapped in 16
    idxflat = np.zeros((NCORES, TOTIDX), np.int16)
    callbase = {}
    off = 0
    for gi, (G, C, blocks) in enumerate(gsizes):
        for w in range(NWIN):
            callbase[(gi, w)] = off
            off += G * C * P
    assert off == TOTIDX
    gi_of_block = np.empty(B, np.int64)
    bi_of_block = np.empty(B, np.int64)
    for gi, (G, C, blocks) in enumerate(gsizes):
        for bi, b in enumerate(blocks):
            gi_of_block[b] = gi
            bi_of_block[b] = bi
    Cb = np.where(np.arange(B) < NBIG, CBIG, CSMALL)
    base_e = np.array([callbase[(gi_of_block[b], w)]
                       for b, w in zip(dbs, ws)], np.int64)
    flatpos = base_e + bi_of_block[dbs] * Cb[dbs] * P + rank
    idxflat[dcs, flatpos] = (rs - ws * WROWS).astype(np.int16)

    idxw = np.zeros((NCORES, P, TOTIDX // 16), np.int16)
    for c in range(NCORES):
        o = 0
        for gi, (G, C, blocks) in enumerate(gsizes):
            ni = G * C * P
            for w in range(NWIN):
                seg = idxflat[c, o:o + ni]
                wrp = seg.reshape(ni // 16, 16).T
                c0 = o // 16
                for g8 in range(8):
                    idxw[c, g8 * 16:(g8 + 1) * 16, c0:c0 + ni // 16] = wrp
                o += ni

    epsdegT = np.full((NCORES, P, B), LN_EPS, np.float32)
    disT = np.zeros((NCORES, P, B), np.float32)
    node_of = np.full((NCORES, SH), -1, np.int64)
    allv = np.arange(n)
    epsdegT[core_of, pos_of, block_of] = LN_EPS * deg[allv]
    disT[core_of, pos_of, block_of] = dis[allv]
    node_of[core_of, block_of * P + pos_of] = allv

    return dict(
        slot=slot, trow=trow, core_of=core_of, node_of=node_of,
        dpos=dpos, idxw=idxw, epsdegT=epsdegT, disT=disT,
        dis=dis, deginv=deginv,
    )


# --------------------------------------------------------------------------
# Device program
# --------------------------------------------------------------------------

def _build(rep=1):
    nc = bacc.Bacc("TRN2", target_bir_lowering=False, debug=False,
                   num_devices=NCORES, num_swdge_queues=4)
    CW = B + B           # epsdegT | disT
    CH = P + 4 * P + P   # ident | emb1 x4 | iota

    xs0 = nc.dram_tensor("xs0", [SLOTS, D], dt.float16, kind="ExternalInput")
    dposT = nc.dram_tensor("dposT", [P, NCH], dt.float32,
                           kind="ExternalInput")
    idx16 = nc.dram_tensor("idx16", [P, TOTIDX // 16], dt.int16,
                           kind="ExternalInput")
    cstF = nc.dram_tensor("cstF", [P, CW], dt.float32, kind="ExternalInput")
    cstH = nc.dram_tensor("cstH", [P, CH], dt.float16, kind="ExternalInput")
    rootf16 = nc.dram_tensor("rootf16", [SH, D], dt.float16,
                             kind="ExternalInput")
    out_sh = nc.dram_tensor("out_sh", [SH, D], dt.float32,
                            kind="ExternalOutput")

    o_epsdeg, o_dis = 0, B
    o_iota = 5 * P

    with tile.TileContext(nc) as tc:
        with (
            tc.tile_pool(name="const", bufs=1) as cpool,
            tc.tile_pool(name="sw", bufs=3) as spool,
            tc.tile_pool(name="gbuf", bufs=3) as gpool,
            tc.tile_pool(name="rpool", bufs=3) as rpool,
            tc.tile_pool(name="og", bufs=3) as opool,
            tc.tile_pool(name="fpool", bufs=6) as fpool,
            tc.tile_pool(name="small", bufs=8) as mpool,
            tc.tile_pool(name="psum", bufs=2, space="PSUM") as psum,
            tc.tile_pool(name="dram", bufs=1, space="DRAM") as dram,
        ):
            xs1_sh = dram.tile([SH, D], dt.float16)

            cb = cpool.tile([P, CW], dt.float32)
            ch = cpool.tile([P, CH], dt.float16)
            dp = cpool.tile([P, NCH], dt.float32)
            ix = cpool.tile([P, TOTIDX // 16], dt.int16)
            ybig = cpool.tile([P, B * D], dt.float16)
            t2big = cpool.tile([P, B * D], dt.float16)
            stash = cpool.tile([P, B * D], dt.float16)
            nc.sync.dma_start(out=cb[:], in_=cstF[:])
            nc.sync.dma_start(out=ch[:], in_=cstH[:])
            nc.sync.dma_start(out=dp[:], in_=dposT[:])
            IXQ = TOTIDX // 64
            for qi, eng in enumerate((nc.sync, nc.scalar, nc.sync,
                                      nc.scalar)):
                eng.dma_start(out=ix[:, qi * IXQ:(qi + 1) * IXQ],
                              in_=idx16[:, qi * IXQ:(qi + 1) * IXQ])
            ident = ch[:, 0:P]

            call_no = 0
            schunk0 = [0]
            for (G, C) in GROUPS:
                schunk0.append(schunk0[-1] + G * NWIN * C)
            calloff = []
            off = 0
            for (G, C) in GROUPS:
                rw = []
                for w in range(NWIN):
                    rw.append(off)
                    off += G * C * P
                calloff.append(rw)

            def _finalize(li, b, bi, ps, xg):
                sm = mpool.tile([P, 1], dt.float32, tag="sm")
                nc.vector.reduce_sum(sm[:], ps[:], axis=mybir.AxisListType.X)
                sq = fpool.tile([P, D], dt.float32, tag="sq")
                ssq = mpool.tile([P, 1], dt.float32, tag="ssq")
                nc.scalar.activation(sq[:], ps[:],
                                     mybir.ActivationFunctionType.Square,
                                     accum_out=ssq[:])
                mu = mpool.tile([P, 1], dt.float32, tag="mu")
                nc.vector.tensor_scalar(out=mu[:], in0=sm[:],
                                        scalar1=1.0 / D, scalar2=None,
                                        op0=mybir.AluOpType.mult)
                m2 = mpool.tile([P, 1], dt.float32, tag="m2")
                nc.vector.tensor_tensor(out=m2[:], in0=mu[:], in1=mu[:],
                                        op=mybir.AluOpType.mult)
                var = mpool.tile([P, 1], dt.float32, tag="var")
                nc.vector.tensor_scalar(out=var[:], in0=ssq[:],
                                        scalar1=1.0 / D,
                                        scalar2=m2[:, 0:1],
                                        op0=mybir.AluOpType.mult,
                                        op1=mybir.AluOpType.subtract)
                std = mpool.tile([P, 1], dt.float32, tag="std")
                nc.scalar.activation(std[:], var[:],
                                     mybir.ActivationFunctionType.Sqrt,
                                     bias=cb[:, o_epsdeg + b:o_epsdeg + b + 1])
                rstd = mpool.tile([P, 1], dt.float32, tag="rstd")
                nc.vector.reciprocal(rstd[:], std[:])
                if li == 0:
                    yb = ybig[:, b * D:(b + 1) * D]
                    nc.vector.tensor_scalar(out=yb, in0=ps[:],
                                            scalar1=mu[:, 0:1],
                                            scalar2=rstd[:, 0:1],
                                            op0=mybir.AluOpType.subtract,
                                            op1=mybir.AluOpType.mult)
                    nc.scalar.activation(xg[:, bi * D:(bi + 1) * D], yb,
                                         mybir.ActivationFunctionType.Relu,
                                         scale=cb[:, o_dis + b:o_dis + b + 1])
                else:
                    nc.vector.tensor_scalar(out=xg[:, bi * D:(bi + 1) * D],
                                            in0=ps[:],
                                            scalar1=mu[:, 0:1],
                                            scalar2=rstd[:, 0:1],
                                            op0=mybir.AluOpType.subtract,
                                            op1=mybir.AluOpType.mult)

            def do_group(srcwin, li, gi, g_b0, phase):
                # phase: "full" (all 4 windows, finalize), "pA" (windows 0,1
                # into a stashed partial), "pB" (windows 2,3 + stash + root,
                # finalize)
                nonlocal call_no
                G, C = GROUPS[gi]
                blocks = list(range(g_b0, g_b0 + G))
                ni = G * C * P
                if phase == "full":
                    wins = (0, 1, 2, 3)
                    sch0, schn = schunk0[gi], G * NWIN * C
                elif phase == "pA":
                    wins = (0, 1)
                    sch0, schn = schunk0[gi], G * 2 * C
                else:
                    wins = (2, 3)
                    sch0, schn = schunk0[gi] + G * 2 * C, G * 2 * C
                st = spool.tile([P, schn * P], dt.float16, tag="st")
                for j in range(schn):
                    nc.vector.tensor_scalar(
                        out=st[:, j * P:(j + 1) * P],
                        in0=ch[:, o_iota:o_iota + P],
                        scalar1=dp[:, sch0 + j:sch0 + j + 1],
                        scalar2=None,
                        op0=mybir.AluOpType.is_equal)
                if phase == "full" and li == 0:
                    rt = rpool.tile([P, G * D], dt.float16, tag="rt")
                    nc.sync.dma_start(
                        out=rt[:].rearrange("p (g d) -> p g d", d=D),
                        in_=rootf16[blocks[0] * P:
                                    (blocks[0] + G) * P, :].rearrange(
                            "(g p) d -> p g d", p=P))
                elif phase == "pB":
                    rt = t2big[:, blocks[0] * D:(blocks[0] + G) * D]
                gts = {}
                for w in wins:
                    gt = gpool.tile([P, ni], dt.float16, tag=f"gt{w}")
                    nc.gpsimd.dma_gather(
                        out_ap=gt[:].rearrange("p (n d) -> p n d", d=D),
                        in_ap=srcwin(w),
                        idxs_ap=ix[:, calloff[gi][w] // 16:
                                   (calloff[gi][w] + ni) // 16],
                        num_idxs=ni,
                        num_idxs_reg=ni,
                        elem_size=D,
                        queue_num=call_no % 4,
                    )
                    call_no += 1
                    gts[w] = gt
                pss = [psum.tile([P, D], dt.float32, space="PSUM",
                                 tag=f"ps{bi}", name=f"ps{bi}")
                       for bi in range(G)]
                k = 0
                for wi, w in enumerate(wins):
                    for j in range(C):
                        for bi in range(G):
                            last = (phase == "pA" and wi == 1 and j == C - 1)
                            nc.tensor.matmul(
                                out=pss[bi][:],
                                lhsT=st[:, k * P:(k + 1) * P],
                                rhs=gts[w][:, (bi * C + j) * P:
                                           (bi * C + j + 1) * P],
                                start=(wi == 0 and j == 0),
                                stop=last,
                            )
                            k += 1
                if phase == "pA":
                    for bi, b in enumerate(blocks):
                        nc.scalar.activation(
                            stash[:, b * D:(b + 1) * D], pss[bi][:],
                            mybir.ActivationFunctionType.Copy)
                    return
                if phase == "pB":
                    for bi, b in enumerate(blocks):
                        nc.tensor.matmul(
                            out=pss[bi][:], lhsT=ident,
                            rhs=stash[:, b * D:(b + 1) * D],
                            start=False, stop=False,
                        )
                for bi in range(G):
                    nc.tensor.matmul(
                        out=pss[bi][:], lhsT=ident,
                        rhs=rt[:, bi * D:(bi + 1) * D],
                        start=False, stop=True,
                    )
                if li == 0:
                    xg = opool.tile([P, G * D], dt.float16, tag="xg")
                else:
                    xg = opool.tile([P, G * D], dt.float32, tag="og")
                for bi, b in enumerate(blocks):
                    _finalize(li, b, bi, pss[bi], xg)
                od = xs1_sh if li == 0 else out_sh
                nc.sync.dma_start(
                    out=od[blocks[0] * P:
                           (blocks[0] + G) * P, :].rearrange(
                        "(g p) d -> p g d", p=P),
                    in_=xg[:].rearrange("p (g d) -> p g d", d=D))

            def root_prep():
                b0 = 0
                for gi, (G, C) in enumerate(GROUPS):
                    blocks = list(range(b0, b0 + G))
                    b0 += G
                    t1a = rpool.tile([P, G * D], dt.float16, tag="t1a")
                    nc.vector.tensor_scalar(
                        out=t1a[:],
                        in0=ybig[:, blocks[0] * D:(blocks[0] + G) * D],
                        scalar1=0.0, scalar2=None,
                        op0=mybir.AluOpType.max)
                    t1 = rpool.tile([P, G * D], dt.float16, tag="t1")
                    nc.vector.tensor_tensor(
                        out=t1[:], in0=t1a[:],
                        in1=ch[:, P:P + G * D],
                        op=mybir.AluOpType.add)
                    for bi, b in enumerate(blocks):
                        nc.scalar.activation(
                            t2big[:, b * D:(b + 1) * D],
                            t1[:, bi * D:(bi + 1) * D],
                            mybir.ActivationFunctionType.Relu,
                            scale=cb[:, o_dis + b:o_dis + b + 1])

            def src0(w):
                return xs0[w * WROWS:(w + 1) * WROWS, :]

            for ri in range(rep):
                # fresh Shared AllGather outputs per rep (a Shared tensor
                # may only be written by a single collective)
                xs1_h = [dram.tile([NCORES * HROWS, D], dt.float16,
                                   addr_space="Shared", tag=f"xs1h{h}r{ri}",
                                   name=f"xs1h{h}r{ri}")
                         for h in (0, 1)]

                def src1(w):
                    return xs1_h[w // 2][(w % 2) * WROWS:
                                         (w % 2 + 1) * WROWS, :]

                b0 = 0
                for gi in range(len(GROUPS)):
                    do_group(src0, 0, gi, b0, "full")
                    b0 += GROUPS[gi][0]
                    if gi == NGRP_H0 - 1:
                        nc.gpsimd.collective_compute(
                            "AllGather", mybir.AluOpType.bypass,
                            replica_groups=[list(range(NCORES))],
                            ins=[xs1_sh[0:HROWS, :].opt()],
                            outs=[xs1_h[0][:, :].opt()],
                        )
                nc.gpsimd.collective_compute(
                    "AllGather", mybir.AluOpType.bypass,
                    replica_groups=[list(range(NCORES))],
                    ins=[xs1_sh[HROWS:SH, :].opt()],
                    outs=[xs1_h[1][:, :].opt()],
                )
                root_prep()
                b0 = 0
                for gi in range(len(GROUPS)):
                    do_group(src1, 1, gi, b0, "pA")
                    b0 += GROUPS[gi][0]
                # keep pB's gathers out of the Pool stream until pA's are
                # issued: the scheduler otherwise hoists pB gathers (which
                # block on AllGather #2) ahead of ready pA work
                with tc.tile_wait_until(1.0 * (ri + 1)):
                    b0 = 0
                    for gi in range(len(GROUPS)):
                        do_group(src1, 1, gi, b0, "pB")
                        b0 += GROUPS[gi][0]
    nc.finalize()
    return nc


# --------------------------------------------------------------------------
# Entry points
# --------------------------------------------------------------------------

def prepare(in_feat, edge_index, root_emb0, root_emb1,
            ln0_g, ln0_b, ln1_g, ln1_b, rep=1):
    in_feat = np.asarray(in_feat, dtype=np.float32)
    edge_index = np.asarray(edge_index)
    n, d = in_feat.shape
    e = edge_index.shape[1]
    assert d == D and n <= SLOTS

    assert (np.all(np.asarray(ln0_g) == 1.0)
            and np.all(np.asarray(ln0_b) == 0.0)
            and np.all(np.asarray(ln1_g) == 1.0)
            and np.all(np.asarray(ln1_b) == 0.0)), \
        "identity LayerNorm affine assumed"

    pl = _plan(edge_index, n, e)

    if rep not in _CACHE:
        _CACHE[rep] = _build(rep=rep)
    nc = _CACHE[rep]

    dis, deginv = pl["dis"], pl["deginv"]
    slot = pl["slot"]

    xs0 = np.zeros((SLOTS, D), np.float16)
    xs0[pl["trow"]] = (dis[:, None]
                       * np.maximum(in_feat, 0.0)).astype(np.float16)

    emb0v = np.asarray(root_emb0, np.float64).reshape(1, D)
    emb1 = np.asarray(root_emb1, np.float32).reshape(1, D)

    identm = np.eye(P, dtype=np.float16)
    iotam = np.broadcast_to(np.arange(P, dtype=np.float16), (P, P))
    csth = np.concatenate(
        [identm] + [np.broadcast_to(emb1, (P, D))] * 4 + [iotam], axis=1
    ).astype(np.float16)

    in_maps = []
    for c in range(NCORES):
        node_of = pl["node_of"][c]
        rootfc = np.zeros((SH, D), np.float16)
        valid = node_of >= 0
        nv = node_of[valid]
        # root term pre-divided by dis[dst]: relu(x+emb)/deg / dis
        # = relu(x+emb)*dis
        rootfc[valid] = np.maximum(
            (in_feat[nv].astype(np.float64) + emb0v)
            * dis[nv][:, None], 0.0).astype(np.float16)
        cst = np.concatenate([
            pl["epsdegT"][c], pl["disT"][c],
        ], axis=1).astype(np.float32)
        in_maps.append({
            "xs0": xs0, "dposT": pl["dpos"][c], "idx16": pl["idxw"][c],
            "cstF": cst, "cstH": csth, "rootf16": rootfc,
        })

    def post(results):
        out = np.zeros((n, D), np.float32)
        for c in range(NCORES):
            node_of = pl["node_of"][c]
            valid = node_of >= 0
            out[node_of[valid]] = results[c]["out_sh"][valid]
        return out

    return nc, in_maps, post


def kernel(in_feat, edge_index, root_emb0, root_emb1,
           ln0_g, ln0_b, ln1_g, ln1_b):
    nc, in_maps, post = prepare(in_feat, edge_index, root_emb0, root_emb1,
                                ln0_g, ln0_b, ln1_g, ln1_b)
    res = run_bass_kernel_spmd(nc, in_maps, core_ids=list(range(NCORES)))
    return post(res.results)

